# revision 1
# baseline (speedup 1.0000x reference)
"""Spikformer block (Q/K/V linear+BN+{LIF,ReLU,ternary} -> headwise linear attention
-> O linear+BN+LIF) on 8 TRN2 NeuronCores, data-parallel over batch.

Self-contained: hardcodes shapes; builds one SPMD Bass program; shards batch
across 8 cores; gathers/transposes on host.

Key algebra / precision plan:
  - attention has no softmax -> (q k^T) v reassociated as q (k^T v); per-head
    k^T v is 64x64, ~8x fewer MACs and no 512x512 attn matrix to evict.
  - all BatchNorms are eval-mode affine: folded into per-channel scale/bias on
    host.
  - matmul dtypes tuned per-path to the LIF/ternary flip sensitivity
    (measured on the reference inputs; fp32r rounds operands to 12-bit
    significands (RNE, verified on silicon) and streams at 1 cyc/row for
    free dim >= 256):
      * Q linear: 1 fp32r pass
      * K linear: 1 fp32r pass   (errors average out in the k^T v sum)
      * V linear: 3 passes (fp32r + bf16 x-residual + fp32r w-residual);
        ternary thresholding is the most flip-sensitive spot.
      * O linear: 1 fp32r pass
      * attention: fp16 single-pass (q binary and v ternary are exact in
        fp16; k/kv quantization adds ~1e-4 relative, tolerable).
  - k^T v computed as one 128-wide matmul per token chunk covering 2 heads;
    the cross-head off-diagonal blocks are garbage but never read: only the
    two diagonal 64x64 blocks are copied into pre-zeroed kv tiles.
  - phases interleaved per x-quarter (K,Q,V) and per-timestep (attention/O)
    so DMA and eviction chains overlap PE work.
"""
import sys
for p in ("/opt/trn_rl_repo",):
    if p not in sys.path:
        sys.path.insert(0, p)

import numpy as np
import ml_dtypes

import concourse.bass as bass
import concourse.bacc as bacc
import concourse.mybir as mybir
import concourse.tile as tile
from concourse.bass_utils import run_bass_kernel_spmd

B, T, L, D, H = 8, 4, 512, 512, 8
HD = D // H
NT = T * L            # 2048 tokens per core
P = 128
EC = D // P           # 4 e-chunks
DC = D // P           # 4 d-chunks
TC = NT // P          # 16 token chunks
EPS = 1e-5
F32 = mybir.dt.float32
F32R = mybir.dt.float32r
BF16 = mybir.dt.bfloat16
FP16 = mybir.dt.float16
F = mybir.ActivationFunctionType
ALU = mybir.AluOpType

_PROGRAM_CACHE = {}
_last_in_maps = None


def _build_program():
    if "nc" in _PROGRAM_CACHE:
        return _PROGRAM_CACHE["nc"]

    nc = bacc.Bacc("TRN2", target_bir_lowering=False, debug=False, num_devices=8)

    dram = {}
    # x and weights are packed on host into [128, n*512] layouts so each
    # SBUF tile fills with a single DMA (HWDGE slots are ~630ns each).
    dram["xp"] = nc.dram_tensor("xp", [P, T * DC * L], F32R, kind="ExternalInput")
    dram["xlp"] = nc.dram_tensor("xlp", [P, T * DC * L], BF16, kind="ExternalInput")
    for w in ("wq", "wk", "wv", "wvl32", "wo"):
        dram[w] = nc.dram_tensor(w, [P, DC * D], F32R, kind="ExternalInput")
    dram["wv8"] = nc.dram_tensor("wv8", [P, DC * D], BF16, kind="ExternalInput")
    dram["cpack"] = nc.dram_tensor("cpack", [P, 16], F32, kind="ExternalInput")
    dram["obrow"] = nc.dram_tensor("obrow", [1, 2 * D + L], F32R, kind="ExternalInput")
    dram["kbb"] = nc.dram_tensor("kbb", [P, D], F32, kind="ExternalInput")
    dram["vthrp"] = nc.dram_tensor("vthrp", [P, 2 * D], F32, kind="ExternalInput")
    out_d = nc.dram_tensor("out", [D, NT], BF16, kind="ExternalOutput")

    with tile.TileContext(nc) as tc_:
        with tc_.tile_pool(name="sb", bufs=1) as sb, \
             tc_.tile_pool(name="sc", bufs=3) as sc, \
             tc_.tile_pool(name="sp8", bufs=6) as sp8, \
             tc_.tile_pool(name="ps", bufs=5, space="PSUM") as ps, \
             tc_.tile_pool(name="pk", bufs=3, space="PSUM") as pk:

            # ---------- persistent SBUF tiles ----------
            # x per quarter, dc-major columns: xq[q4][:, dc*L + t]
            xq = [sb.tile([P, DC * L], F32R, tag="xa", bufs=DC, name=f"xq{i}") for i in range(T)]
            xlq = [sb.tile([P, DC * L], BF16, tag="xl", bufs=DC, name=f"xlq{i}") for i in range(T)]
            wts = {}
            for w in ("wq", "wk", "wv", "wvl32", "wo"):
                wts[w] = sb.tile([P, DC * D], F32R, tag=w, name=w)
            wts["wv8"] = sb.tile([P, DC * D], BF16, tag="wv8", name="wv8")
            qT16 = [sb.tile([P, NT], FP16, tag="qT", bufs=EC, name=f"qT{i}") for i in range(EC)]
            k16 = [sb.tile([P, D], FP16, tag="k16", bufs=TC, name=f"k16_{i}") for i in range(TC)]
            v16 = [sb.tile([P, D], FP16, tag="v16", bufs=TC, name=f"v16_{i}") for i in range(TC)]
            kv16 = [sb.tile([P, P], FP16, tag="kv16", bufs=TC, name=f"kv16_{i}") for i in range(TC)]
            # attention output [e, t] fp32r; reuses the xT buffers (tag "xa"),
            # safe because all xT reads precede phase 2.
            ao = [sb.tile([P, NT], F32R, tag="xa", bufs=DC, name=f"ao{i}") for i in range(DC)]
            memq = [sb.tile([P, L], F32, tag="memq", bufs=EC, name=f"memq{i}") for i in range(EC)]
            obrow = sb.tile([1, 2 * D + L], F32R, tag="obrow")
            onesrow = obrow[:, 2 * D:2 * D + L]
            memo = [sb.tile([P, L], F32, tag="memo", bufs=EC, name=f"memo{i}") for i in range(EC)]
            r3t = [sb.tile([P, L], F32, tag="r3t", bufs=EC, name=f"r3t{i}") for i in range(EC)]
            cpack = sb.tile([P, 16], F32, tag="cst")
            consts = {v_: [cpack[:, (vi * 4 + i):(vi * 4 + i + 1)] for i in range(EC)]
                      for vi, v_ in enumerate(("qs", "qb", "os_", "ob"))}
            kbbt = sb.tile([P, D], F32, tag="kbb")
            kbb = kbbt[:]
            vthrp = sb.tile([P, 2 * D], F32, tag="thr")
            vthr1 = vthrp[:, 0:D]
            vthr2 = vthrp[:, D:2 * D]

            # kv tiles: zero once; only diagonal 64x64 blocks are ever written
            for i in range(TC):
                nc.gpsimd.memset(kv16[i][:], 0.0)

            # ---------- loads ----------
            # single queue (SP), exact consumption order; DMAs on compute
            # queues would steal their sequencers (~667ns per issue).
            W = DC * L
            nc.sync.dma_start(wts["wk"][:, 0:D], dram["wk"][:, 0:D])

            def ldx(q4):
                for hh in range(2):
                    hs = slice(hh * W // 2, (hh + 1) * W // 2)
                    nc.sync.dma_start(xq[q4][:, hs],
                                      dram["xp"][:, q4 * W + hh * W // 2:q4 * W + (hh + 1) * W // 2])

            nc.sync.dma_start(xq[0][:, 0:W // 2], dram["xp"][:, 0:W // 2])
            nc.sync.dma_start(wts["wk"][:, D:DC * D], dram["wk"][:, D:DC * D])
            nc.sync.dma_start(xq[0][:, W // 2:W], dram["xp"][:, W // 2:W])
            nc.sync.dma_start(kbbt[:], dram["kbb"][:])
            ldx(1)
            ldx(2)
            ldx(3)
            nc.sync.dma_start(wts["wq"][:], dram["wq"][:])
            nc.sync.dma_start(cpack[:], dram["cpack"][:])
            for q4 in range(T):
                nc.sync.dma_start(xlq[q4][:], dram["xlp"][:, q4 * W:(q4 + 1) * W])
            nc.sync.dma_start(vthrp[:], dram["vthrp"][:])
            nc.sync.dma_start(wts["wv"][:], dram["wv"][:])
            nc.sync.dma_start(wts["wvl32"][:], dram["wvl32"][:])
            nc.sync.dma_start(wts["wv8"][:], dram["wv8"][:])
            nc.sync.dma_start(wts["wo"][:], dram["wo"][:])
            nc.sync.dma_start(obrow[:], dram["obrow"][:])

            # ---------- phase 1: per x-quarter: K tiles, Q tile, then V of
            # the PREVIOUS quarter (gives the DMA a quarter of slack to land
            # the V weights and xlT residuals) ----
            def emit_v(q4):
                for mc in range(4):
                    tc2 = q4 * 4 + mc
                    mc4 = mc
                    pvv = ps.tile([P, D], F32, tag="mm512", name=f"pv{tc2}")
                    for dc in range(DC):
                        nc.tensor.matmul(pvv[:], xq[q4][:, dc * L + mc4 * P:dc * L + (mc4 + 1) * P],
                                         wts["wv"][:, dc * D:(dc + 1) * D],
                                         start=(dc == 0), stop=False)
                    for dc in range(DC):
                        nc.tensor.matmul(pvv[:], xq[q4][:, dc * L + mc4 * P:dc * L + (mc4 + 1) * P],
                                         wts["wvl32"][:, dc * D:(dc + 1) * D],
                                         start=False, stop=False)
                    for dc in range(DC):
                        nc.tensor.matmul(pvv[:], xlq[q4][:, dc * L + mc4 * P:dc * L + (mc4 + 1) * P],
                                         wts["wv8"][:, dc * D:(dc + 1) * D],
                                         start=False, stop=(dc == DC - 1))
                    t1 = sc.tile([P, D], FP16, tag="t1", name=f"t1_{tc2}")
                    t2 = sc.tile([P, D], FP16, tag="t2", name=f"t2_{tc2}")
                    nc.vector.tensor_tensor(t1[:], pvv[:], vthr1, ALU.is_ge)
                    nc.vector.tensor_tensor(t2[:], pvv[:], vthr2, ALU.is_le)
                    nc.gpsimd.tensor_sub(v16[tc2][:], t1[:], t2[:])

            # all K tiles first (needs only wk + xT), then Q (wq), then V
            for tc2 in range(TC):
                q4k, mc4 = tc2 // 4, tc2 % 4
                pkv = ps.tile([P, D], F32, tag="mm512", name=f"pk{tc2}")
                for dc in range(DC):
                    nc.tensor.matmul(pkv[:], xq[q4k][:, dc * L + mc4 * P:dc * L + (mc4 + 1) * P],
                                     wts["wk"][:, dc * D:(dc + 1) * D],
                                     start=(dc == 0), stop=(dc == DC - 1))
                kf = sc.tile([P, D], F32, tag="kf", name=f"kf{tc2}")
                nc.vector.tensor_tensor(kf[:], pkv[:], kbb, ALU.add)
                nc.scalar.activation(k16[tc2][:], kf[:], F.Relu)

            for ti in range(T):
                xs = slice(ti * L, (ti + 1) * L)
                for ec in range(EC):
                    es = slice(ec * P, (ec + 1) * P)
                    pq = ps.tile([P, L], F32, tag="mm512", name=f"pq{ti}_{ec}")
                    for dc in range(DC):
                        nc.tensor.matmul(pq[:], wts["wq"][:, dc * D + ec * P:dc * D + (ec + 1) * P],
                                         xq[ti][:, dc * L:(dc + 1) * L],
                                         start=(dc == 0), stop=(dc == DC - 1))
                    if ti == 0:
                        nc.scalar.activation(memq[ec][:], pq[:], F.Identity,
                                             bias=consts["qb"][ec], scale=consts["qs"][ec])
                    else:
                        u = sc.tile([P, L], F32, tag="u", name=f"u{ti}_{ec}")
                        nc.scalar.activation(u[:], pq[:], F.Identity,
                                             bias=consts["qb"][ec], scale=consts["qs"][ec])
                        nc.vector.scalar_tensor_tensor(memq[ec][:], memq[ec][:], 0.5, u[:],
                                                       ALU.mult, ALU.add)
                        nc.gpsimd.tensor_sub(memq[ec][:], memq[ec][:],
                                             qT16[ec][:, (ti - 1) * L:ti * L])
                    nc.gpsimd.tensor_scalar(qT16[ec][:, xs], memq[ec][:], 1.0, None, ALU.is_ge)
            for q4 in range(T):
                emit_v(q4)

            # ---------- phase 2+3: attention and O-linear, interleaved per ti
            prev_spk = [None] * EC

            def emit_o(ti, ecs=range(EC)):
                # os is folded into wo on host, so psum holds os*lin; bias ob
                # is added by act (mid steps) or a K=1 matmul (final step).
                xs = slice(ti * L, (ti + 1) * L)
                final = (ti == T - 1)
                for ec in ecs:
                    es = slice(ec * P, (ec + 1) * P)
                    po = ps.tile([P, L], F32, tag="mm512")
                    for dc in range(DC):
                        nc.tensor.matmul(po[:], wts["wo"][:, dc * D + ec * P:dc * D + (ec + 1) * P],
                                         ao[dc][:, xs],
                                         start=(dc == 0), stop=(dc == DC - 1 and not final))
                    if final:
                        nc.tensor.matmul(po[:], obrow[:, es], onesrow,
                                         start=False, stop=False)
                        nc.tensor.matmul(po[:], obrow[:, D + ec * P:D + (ec + 1) * P], onesrow,
                                         start=False, stop=True)
                    spk = sp8.tile([P, L], BF16, tag="ospk")
                    if ti == 0:
                        nc.scalar.activation(memo[ec][:], po[:], F.Identity,
                                             bias=consts["ob"][ec])
                        nc.gpsimd.tensor_scalar(spk[:], memo[ec][:], 1.0, None, ALU.is_ge)
                    elif not final:
                        u = sc.tile([P, L], F32, tag="u")
                        nc.scalar.activation(u[:], po[:], F.Identity,
                                             bias=consts["ob"][ec])
                        nc.vector.scalar_tensor_tensor(memo[ec][:], memo[ec][:], 0.5, u[:],
                                                       ALU.mult, ALU.add)
                        nc.gpsimd.tensor_sub(memo[ec][:], memo[ec][:], prev_spk[ec][:])
                        nc.gpsimd.tensor_scalar(spk[:], memo[ec][:], 1.0, None, ALU.is_ge)
                        if ti == T - 2:
                            # final-step threshold: spk3 = (po3 >= 1 + spk2 - mem2/2)
                            nc.gpsimd.tensor_scalar(r3t[ec][:], memo[ec][:], -0.5, 1.0,
                                                    ALU.mult, ALU.add)
                            nc.gpsimd.tensor_tensor(r3t[ec][:], r3t[ec][:], spk[:], ALU.add)
                    else:
                        # final step: bias already in psum; entire LIF update
                        # folded into one compare against the precomputed r3t
                        nc.vector.tensor_tensor(spk[:], po[:], r3t[ec][:], ALU.is_ge)
                    prev_spk[ec] = spk
                    nc.sync.dma_start(out_d[es, xs], spk[:])

            for ti in range(T):
                xs = slice(ti * L, (ti + 1) * L)
                pkvs = []
                for c in range(EC):          # head pair (2c, 2c+1) == e-chunk c
                    es = slice(c * P, (c + 1) * P)
                    pkv64 = pk.tile([P, P], F32, tag="kv64", name=f"pkv{ti}_{c}")
                    for mc in range(4):
                        tc2 = ti * 4 + mc
                        nc.tensor.matmul(pkv64[:], k16[tc2][:, es], v16[tc2][:, es],
                                         start=(mc == 0), stop=(mc == 3))
                    pkvs.append(pkv64)
                for c in range(EC):
                    kvt = kv16[ti * 4 + c]
                    nc.scalar.copy(kvt[0:HD, 0:HD], pkvs[c][0:HD, 0:HD])
                    nc.scalar.copy(kvt[HD:P, HD:P], pkvs[c][HD:P, HD:P])
                # O-linear of the previous timestep fills PE while the act
                # engine drains the kv copies (first half) and the ao copies
                # (second half).
                if ti > 0:
                    emit_o(ti - 1, (0, 1))
                for c in range(EC):
                    pso = ps.tile([P, L], F32, tag="mm512", name=f"pso{ti}_{c}")
                    nc.tensor.matmul(pso[:], kv16[ti * 4 + c][:], qT16[c][:, xs],
                                     start=True, stop=True)
                    nc.vector.tensor_copy(ao[c][:, xs], pso[:])
                if ti > 0:
                    emit_o(ti - 1, (2, 3))
            emit_o(T - 1)

    nc.compile()
    _PROGRAM_CACHE["nc"] = nc
    return nc


def _rne11(a):
    """Round float32 array to 12-bit significands (RNE) - replicates the PE's
    fp32r operand rounding exactly (verified on hardware)."""
    m, e = np.frexp(np.asarray(a, np.float32).astype(np.float64))
    return np.ldexp(np.rint(m * 4096.0) / 4096.0, e).astype(np.float32)


def _bf16(a):
    return np.asarray(a, np.float32).astype(ml_dtypes.bfloat16)


def kernel(**inputs):
    nc = _build_program()

    f64 = np.float64
    x = np.asarray(inputs["x"], np.float32)

    def bn_fold(g, b_, rm, rv):
        s = (g.astype(f64) / np.sqrt(rv.astype(f64) + EPS))
        bias = b_.astype(f64) - rm.astype(f64) * s
        return s, bias

    sq, bq = bn_fold(inputs["q_g"], inputs["q_b"], inputs["q_rm"], inputs["q_rv"])
    sk, bk = bn_fold(inputs["k_g"], inputs["k_b"], inputs["k_rm"], inputs["k_rv"])
    sv, bv = bn_fold(inputs["v_g"], inputs["v_b"], inputs["v_rm"], inputs["v_rv"])
    so, bo = bn_fold(inputs["o_g"], inputs["o_b"], inputs["o_rm"], inputs["o_rv"])
    C = HD ** -0.5
    # o path: out = bn(lin + o_bias) -> bias' = (o_bias - rm)*s + b
    bo = bo + inputs["o_bias"].astype(f64) * so

    wq = np.ascontiguousarray(inputs["q_w"].astype(f64).T).astype(np.float32)
    wk = np.ascontiguousarray((inputs["k_w"].astype(f64) * (C * sk)[:, None]).T).astype(np.float32)
    wv = np.ascontiguousarray((inputs["v_w"].astype(f64) * sv[:, None]).T).astype(np.float32)
    wo = np.ascontiguousarray(inputs["o_w"].astype(f64).T).astype(np.float32)
    kb_fold = (C * bk).astype(np.float32)
    vb_fold = bv.astype(np.float32)

    def wpack(w):
        return np.ascontiguousarray(np.concatenate(
            [w[dc * P:(dc + 1) * P, :] for dc in range(DC)], axis=1))

    wvl32 = wv - _rne11(wv)
    shared = {
        "wq": wpack(wq), "wk": wpack(wk), "wv": wpack(wv),
        "wo": wpack(wo * so.astype(np.float32)[None, :]),
        "obrow": np.concatenate([_rne11(bo.astype(np.float32)),
                                 bo.astype(np.float32) - _rne11(bo.astype(np.float32)),
                                 np.ones(L, np.float32)]).reshape(1, 2 * D + L),
        "wvl32": wpack(wvl32),
        "wv8": _bf16(wpack(wv)),
        "cpack": np.stack([v.astype(np.float32).reshape(EC, P).T.reshape(P, EC)
                           for v in (sq, bq, so, bo)], axis=1).reshape(P, 16),
        "kbb": np.ascontiguousarray(np.broadcast_to(kb_fold[None, :], (P, D))),
        "vthrp": np.ascontiguousarray(np.concatenate([
            np.broadcast_to((np.float32(1.0) - vb_fold)[None, :], (P, D)),
            np.broadcast_to((np.float32(-1.0) - vb_fold)[None, :], (P, D)),
        ], axis=1)),
    }

    in_maps = []
    for b in range(B):
        xT = x[b].reshape(NT, D).T                         # (D, NT) f32
        xl = _bf16(xT - _rne11(xT))
        # pack quarter-major, dc-major: xp[:, (q4*DC + dc)*L + t] = xT[dc-chunk, q4-quarter]
        xp = np.concatenate([xT[dc * P:(dc + 1) * P, q4 * L:(q4 + 1) * L]
                             for q4 in range(T) for dc in range(DC)], axis=1)
        xlp = np.concatenate([xl[dc * P:(dc + 1) * P, q4 * L:(q4 + 1) * L]
                              for q4 in range(T) for dc in range(DC)], axis=1)
        m = dict(shared)
        m["xp"] = np.ascontiguousarray(xp)
        m["xlp"] = np.ascontiguousarray(xlp)
        in_maps.append(m)

    global _last_in_maps
    _last_in_maps = in_maps
    res = run_bass_kernel_spmd(nc, in_maps, core_ids=list(range(B)))
    outs = []
    for b in range(B):
        oT = res.results[b]["out"]                    # (D, NT) bf16
        outs.append(oT.reshape(D, T, L).transpose(1, 2, 0))
    return np.stack(outs).astype(np.float32)


if __name__ == "__main__":
    import importlib.util
    spec = importlib.util.spec_from_file_location("reference", "/root/problem/reference.py")
    ref = importlib.util.module_from_spec(spec)
    spec.loader.exec_module(ref)
    inp = {k: np.asarray(v) for k, v in ref.setup_inputs().items()}
    exp = np.asarray(ref.reference(**inp))
    act = kernel(**inp)
    rel = np.linalg.norm(act - exp) / np.linalg.norm(exp)
    print("flips:", int(np.sum(act != exp)), "/", exp.size)
    print("Relative error:", rel)



# revision 31
# speedup vs baseline: 1.3145x; 1.3145x over previous
"""Spikformer block (Q/K/V linear+BN+{LIF,ReLU,ternary} -> headwise linear attention
-> O linear+BN+LIF) on 8 TRN2 NeuronCores, data-parallel over batch.

Self-contained: hardcodes shapes; builds one SPMD Bass program; shards batch
across 8 cores; gathers/transposes on host.

Key algebra / precision plan:
  - attention has no softmax -> (q k^T) v reassociated as q (k^T v); per-head
    k^T v is 64x64, ~8x fewer MACs and no 512x512 attn matrix to evict.
  - all BatchNorms are eval-mode affine: folded into per-channel scale/bias on
    host.
  - matmul dtypes tuned to the LIF/ternary flip sensitivity against the
    rel-err budget (fp32r rounds operands to 12-bit significands (RNE) and
    streams at 1 cyc/row for free dim >= 256; flip counts per precision
    config measured with a bit-faithful numpy model against the reference):
      * Q linear: 2 passes (fp32r main + bf16 x-residual); the q-LIF
        threshold is the second most flip-sensitive spot, and the x-rounding
        term dominates its flips.
      * K linear: 1 fp32r pass (errors wash out in the k^T v sum)
      * V linear: 1 fp32r pass (ternary threshold flips stay in budget)
      * O linear: 1 fp32r pass
      * attention: fp16 throughout (q binary and v ternary are exact in
        fp16; the k/kv quantization adds ~1e-4 relative, tolerable).
  - O-path bias is folded into the act-engine bias adds (steps 0..2) and into
    the precomputed final-step threshold r3t (last step) - no bias matmuls.
  - k^T v computed as one 128-wide matmul per token chunk covering 2 heads;
    the cross-head off-diagonal blocks are garbage but never read: only the
    two diagonal 64x64 blocks are copied into pre-zeroed kv tiles. kv runs
    one quarter behind V so the V thresholds (DVE) stay off the critical
    path, and the copies land on the act engine which idles during V.
  - LIF membrane updates run as in-place DVE scalar_tensor_tensor ops; Pool
    only computes spikes (is_ge), keeping every engine under the PE time.
"""
import sys
for p in ("/opt/trn_rl_repo",):
    if p not in sys.path:
        sys.path.insert(0, p)

import numpy as np
import ml_dtypes

import concourse.bass as bass
import concourse.bacc as bacc
import concourse.mybir as mybir
import concourse.tile as tile
from concourse.bass_utils import run_bass_kernel_spmd

B, T, L, D, H = 8, 4, 512, 512, 8
HD = D // H
NT = T * L            # 2048 tokens per core
P = 128
EC = D // P           # 4 e-chunks
DC = D // P           # 4 d-chunks
TC = NT // P          # 16 token chunks
EPS = 1e-5
F32 = mybir.dt.float32
F32R = mybir.dt.float32r
BF16 = mybir.dt.bfloat16
FP16 = mybir.dt.float16
FP8 = mybir.dt.float8e5
FP8_SCALE = 32.0
F = mybir.ActivationFunctionType
ALU = mybir.AluOpType

_PROGRAM_CACHE = {}
_last_in_maps = None

# structural scheduling choices (CoreSim-swept)
SCHED = {
    "v3_first": False,     # emit V(3) before Q(3) in the last phase-1 window
    "kv3": "ph1",          # 'ph1' (end of phase 1) or 'ti0' (in phase-2 ti0)
    "po_first": False,     # emit_o(ti-1) before pso(ti) in phase 2
    "ao_ti0_act": False,   # ti0 ao copies all on ACT
}


def _build_program():
    if "nc" in _PROGRAM_CACHE:
        return _PROGRAM_CACHE["nc"]

    nc = bacc.Bacc("TRN2", target_bir_lowering=False, debug=False, num_devices=8)

    dram = {}
    # x and weights are packed on host into [128, n*512] layouts so each
    # SBUF tile fills with a single DMA (HWDGE slots are ~630ns each).
    dram["xp"] = nc.dram_tensor("xp", [P, T * DC * L], F32R, kind="ExternalInput")
    dram["xlp"] = nc.dram_tensor("xlp", [P, T * DC, L], FP8, kind="ExternalInput")
    for w in ("wq", "wk", "wv", "wo"):
        dram[w] = nc.dram_tensor(w, [P, DC * D], F32R, kind="ExternalInput")
    dram["wq8"] = nc.dram_tensor("wq8", [P, DC, D], FP8, kind="ExternalInput")
    dram["cpack"] = nc.dram_tensor("cpack", [P, 24], F32, kind="ExternalInput")
    dram["kbb"] = nc.dram_tensor("kbb", [P, D], F32, kind="ExternalInput")
    dram["vthrp"] = nc.dram_tensor("vthrp", [P, 2 * D], F32, kind="ExternalInput")
    out_d = nc.dram_tensor("out", [D, NT], BF16, kind="ExternalOutput")

    with tile.TileContext(nc) as tc_:
        with tc_.tile_pool(name="sb", bufs=1) as sb, \
             tc_.tile_pool(name="sc", bufs=3) as sc, \
             tc_.tile_pool(name="sp8", bufs=6) as sp8, \
             tc_.tile_pool(name="ps", bufs=5, space="PSUM") as ps, \
             tc_.tile_pool(name="pk", bufs=3, space="PSUM") as pk:

            # ---------- persistent SBUF tiles ----------
            # x per quarter, dc-major columns: xq[q4][:, dc*L + t]
            xq = [sb.tile([P, DC * L], F32R, tag="xa", bufs=DC, name=f"xq{i}") for i in range(T)]
            xlq = [sb.tile([P, DC, L], FP8, tag="xl", bufs=DC, name=f"xlq{i}") for i in range(T)]
            wts = {}
            for w in ("wq", "wk", "wv", "wo"):
                wts[w] = sb.tile([P, DC * D], F32R, tag=w, name=w)
            wts["wq8"] = sb.tile([P, DC, D], FP8, tag="wq8", name="wq8")
            qT = [sb.tile([P, NT], FP16, tag="qT", bufs=EC, name=f"qT{i}") for i in range(EC)]
            k16 = [sb.tile([P, D], FP16, tag="k16", bufs=TC, name=f"k16_{i}") for i in range(TC)]
            v16 = [sb.tile([P, D], FP16, tag="v16", bufs=TC, name=f"v16_{i}") for i in range(TC)]
            kvt = [sb.tile([P, P], FP16, tag="kvt", bufs=TC, name=f"kvt_{i}") for i in range(TC)]
            # attention output [e, t] fp32r; reuses the xq buffers (tag "xa"),
            # safe because all xq reads precede phase 2.
            ao = [sb.tile([P, NT], F32R, tag="xa", bufs=DC, name=f"ao{i}") for i in range(DC)]
            memq = [sb.tile([P, L], F32, tag="memq", bufs=EC, name=f"memq{i}") for i in range(EC)]
            memo = [sb.tile([P, L], F32, tag="memo", bufs=EC, name=f"memo{i}") for i in range(EC)]
            # r3t is only live in phase 2; reuse the memq buffers (dead after
            # phase 1b) by allocating on the same tag ring.
            r3t = [sb.tile([P, L], F32, tag="memq", bufs=EC, name=f"r3t{i}") for i in range(EC)]
            cpack = sb.tile([P, 24], F32, tag="cst")
            consts = {v_: [cpack[:, (vi * 4 + i):(vi * 4 + i + 1)] for i in range(EC)]
                      for vi, v_ in enumerate(("qs", "qb", "othr0", "othr1", "othr2", "or3c"))}
            kbbt = sb.tile([P, D], F32, tag="kbb")
            kbb = kbbt[:]
            vthrp = sb.tile([P, 2 * D], F32, tag="thr")
            vthr1 = vthrp[:, 0:D]
            vthr2 = vthrp[:, D:2 * D]

            # ---------- loads ----------
            # head: first K matmul needs full wk + full xq0; split them across
            # four DMA queues (SP/act/pool/vector) so both land ~2x sooner.
            # Everything else streams on the SP queue in consumption order.
            W = DC * L
            nc.sync.dma_start(wts["wk"][:, 0:2 * D], dram["wk"][:, 0:2 * D])
            nc.scalar.dma_start(wts["wk"][:, 2 * D:DC * D], dram["wk"][:, 2 * D:DC * D])
            nc.gpsimd.dma_start(xq[0][:, 0:W // 2], dram["xp"][:, 0:W // 2])
            nc.sync.dma_start(xq[0][:, W // 2:W], dram["xp"][:, W // 2:W])

            # kv tiles: zero once; only diagonal 64x64 blocks are ever written
            for i in range(TC):
                nc.gpsimd.memset(kvt[i][:], 0.0)

            def ldx(q4, lo=False):
                if lo:
                    nc.sync.dma_start(xlq[q4][:], dram["xlp"][:, q4 * DC:(q4 + 1) * DC, :])
                    return
                for hh in range(2):
                    nc.sync.dma_start(xq[q4][:, hh * W // 2:(hh + 1) * W // 2],
                                      dram["xp"][:, q4 * W + hh * W // 2:q4 * W + (hh + 1) * W // 2])

            # Q-phase operands ride the act queue (idle until the K relus),
            # landing well before the K->Q boundary; everything else streams
            # on SP behind the K-phase x quarters.
            nc.scalar.dma_start(wts["wq"][:], dram["wq"][:])
            nc.scalar.dma_start(wts["wq8"][:], dram["wq8"][:])
            nc.scalar.dma_start(xlq[0][:], dram["xlp"][:, 0:DC, :])
            nc.sync.dma_start(kbbt[:], dram["kbb"][:])
            ldx(1)
            ldx(2)
            ldx(3)
            nc.sync.dma_start(cpack[:], dram["cpack"][:])
            nc.sync.dma_start(wts["wv"][:], dram["wv"][:])
            ldx(1, lo=True)
            nc.sync.dma_start(vthrp[:], dram["vthrp"][:])
            ldx(2, lo=True)
            ldx(3, lo=True)
            nc.sync.dma_start(wts["wo"][:], dram["wo"][:])

            # ---------- phase 1a: K tiles (relu'd, fp16) ----------
            for tc2 in range(TC):
                q4k, mc4 = tc2 // 4, tc2 % 4
                pkv = ps.tile([P, D], F32, tag="mm512", name=f"pk{tc2}")
                for dc in range(DC):
                    nc.tensor.matmul(pkv[:], xq[q4k][:, dc * L + mc4 * P:dc * L + (mc4 + 1) * P],
                                     wts["wk"][:, dc * D:(dc + 1) * D],
                                     start=(dc == 0), stop=(dc == DC - 1))
                kf = sc.tile([P, D], F32, tag="kf", name=f"kf{tc2}")
                nc.vector.tensor_tensor(kf[:], pkv[:], kbb, ALU.add)
                nc.scalar.activation(k16[tc2][:], kf[:], F.Relu)

            # ---------- phase 1b: interleaved Q (2-pass, LIF over T) and V
            # quarters. A Q+V window has ~11.1us of PE work against ~7.1us of
            # DVE work, absorbing the V thresholds' DVE deficit; the kv
            # matmuls trail the V quarter by one window so the v16 threshold
            # chains are never on the PE critical path.
            def emit_q(ti):
                xs = slice(ti * L, (ti + 1) * L)
                for ec in range(EC):
                    pq = ps.tile([P, L], F32, tag="mm512", name=f"pq{ti}_{ec}")
                    for dc in range(DC):
                        nc.tensor.matmul(pq[:], wts["wq"][:, dc * D + ec * P:dc * D + (ec + 1) * P],
                                         xq[ti][:, dc * L:(dc + 1) * L],
                                         start=(dc == 0), stop=False)
                    for dcp in (0, 2):
                        nc.tensor.matmul(pq[:], wts["wq8"][:, dcp:dcp + 2, ec * P:(ec + 1) * P],
                                         xlq[ti][:, dcp:dcp + 2, :],
                                         start=False, stop=(dcp == 2),
                                         perf_mode=mybir.MatmulPerfMode.DoubleRow)
                    if ti == 0:
                        nc.scalar.activation(memq[ec][:], pq[:], F.Identity,
                                             bias=consts["qb"][ec], scale=consts["qs"][ec])
                    else:
                        u = sc.tile([P, L], F32, tag="u", name=f"u{ti}_{ec}")
                        nc.scalar.activation(u[:], pq[:], F.Identity,
                                             bias=consts["qb"][ec], scale=consts["qs"][ec])
                        nc.vector.scalar_tensor_tensor(memq[ec][:], memq[ec][:], 0.5, u[:],
                                                       ALU.mult, ALU.add)
                        nc.gpsimd.tensor_sub(memq[ec][:], memq[ec][:],
                                             qT[ec][:, (ti - 1) * L:ti * L])
                    nc.gpsimd.tensor_scalar(qT[ec][:, xs], memq[ec][:], 1.0, None, ALU.is_ge)

            def emit_v(q4):
                for mc in range(4):
                    tc2 = q4 * 4 + mc
                    pvv = ps.tile([P, D], F32, tag="mm512", name=f"pv{tc2}")
                    for dc in range(DC):
                        nc.tensor.matmul(pvv[:], xq[q4][:, dc * L + mc * P:dc * L + (mc + 1) * P],
                                         wts["wv"][:, dc * D:(dc + 1) * D],
                                         start=(dc == 0), stop=(dc == DC - 1))
                    t1 = sc.tile([P, D], FP16, tag="t1", name=f"t1_{tc2}")
                    t2 = sc.tile([P, D], FP16, tag="t2", name=f"t2_{tc2}")
                    nc.vector.tensor_tensor(t1[:], pvv[:], vthr1, ALU.is_ge)
                    nc.vector.tensor_tensor(t2[:], pvv[:], vthr2, ALU.is_le)
                    nc.gpsimd.tensor_sub(v16[tc2][:], t1[:], t2[:])

            def emit_kv(ti):
                # head pair (2c, 2c+1) == e-chunk c; diagonal 64x64 blocks only
                pkvs = []
                for c in range(EC):
                    es = slice(c * P, (c + 1) * P)
                    pkv64 = pk.tile([P, P], F32, tag="kv64", name=f"pkv{ti}_{c}")
                    for mc in range(4):
                        tc2 = ti * 4 + mc
                        nc.tensor.matmul(pkv64[:], k16[tc2][:, es], v16[tc2][:, es],
                                         start=(mc == 0), stop=(mc == 3))
                    pkvs.append(pkv64)
                for c in range(EC):
                    kv = kvt[ti * 4 + c]
                    nc.scalar.copy(kv[0:HD, 0:HD], pkvs[c][0:HD, 0:HD])
                    nc.scalar.copy(kv[HD:P, HD:P], pkvs[c][HD:P, HD:P])

            for i in range(T):
                # last window runs V before Q so V(3)'s DVE threshold chains
                # drain during Q(3) instead of crowding the phase-2 entry
                if i == T - 1 and SCHED["v3_first"]:
                    emit_v(i)
                    emit_q(i)
                else:
                    emit_q(i)
                    emit_v(i)
                if i > 0:
                    emit_kv(i - 1)
            if SCHED["kv3"] == "ph1":
                emit_kv(T - 1)

            # ---------- phase 2: attention q(kv) and O-linear per ti --------
            prev_spk = [None] * EC

            def emit_o(ti, ecs=range(EC)):
                # wo is pre-scaled by the BN scale on host. The per-step bias
                # ob is never added: the membrane is tracked as
                #   m~_t = mem_t - c_t,   c_t = sum_{s<=t} 0.5^(t-s) ob
                # so updates are pure stt ops reading the PSUM po directly,
                # and spikes compare against per-channel thresholds 1 - c_t.
                xs = slice(ti * L, (ti + 1) * L)
                final = (ti == T - 1)
                for ec in ecs:
                    po = ps.tile([P, L], F32, tag="mm512")
                    for dc in range(DC):
                        nc.tensor.matmul(po[:], wts["wo"][:, dc * D + ec * P:dc * D + (ec + 1) * P],
                                         ao[dc][:, xs],
                                         start=(dc == 0), stop=(dc == DC - 1))
                    spk = sp8.tile([P, L], BF16, tag="ospk")
                    if ti == 0:
                        # m~0 = po; spike straight off PSUM, then build the
                        # reset-applied state memo = po - 2 spk in one stt
                        nc.vector.tensor_scalar(spk[:], po[:], consts["othr0"][ec], None,
                                                ALU.is_ge)
                        nc.vector.scalar_tensor_tensor(memo[ec][:], spk[:], -2.0,
                                                       po[:], ALU.mult, ALU.add)
                    elif not final:
                        nc.vector.scalar_tensor_tensor(memo[ec][:], memo[ec][:], 0.5,
                                                       po[:], ALU.mult, ALU.add)
                        thr = consts["othr1" if ti == 1 else "othr2"][ec]
                        nc.gpsimd.tensor_scalar(spk[:], memo[ec][:], thr, None, ALU.is_ge)
                        if ti == T - 2:
                            # r3t = (1 - c3) - m~2/2 + spk2, all on Pool
                            nc.gpsimd.tensor_scalar(r3t[ec][:], memo[ec][:], -0.5,
                                                    consts["or3c"][ec], ALU.mult, ALU.add)
                            nc.gpsimd.tensor_tensor(r3t[ec][:], r3t[ec][:], spk[:], ALU.add)
                        else:
                            # reset in place: memo -= 2 spk; split engines
                            if ec < 2:
                                nc.gpsimd.tensor_sub(memo[ec][:], memo[ec][:], spk[:])
                                nc.gpsimd.tensor_sub(memo[ec][:], memo[ec][:], spk[:])
                            else:
                                nc.vector.scalar_tensor_tensor(memo[ec][:], spk[:], -2.0,
                                                               memo[ec][:], ALU.mult, ALU.add)
                    else:
                        # final step: entire LIF update folded into one compare
                        # against the precomputed r3t (which includes the bias);
                        # done in halves so each half's out DMA starts sooner
                        for h in range(2):
                            hl = slice(h * L // 2, (h + 1) * L // 2)
                            hx = slice(ti * L + h * L // 2, ti * L + (h + 1) * L // 2)
                            nc.vector.tensor_tensor(spk[:, hl], po[:, hl],
                                                    r3t[ec][:, hl], ALU.is_ge)
                            dq = nc.sync if h == 0 else nc.gpsimd
                            dq.dma_start(out_d[ec * P:(ec + 1) * P, hx], spk[:, hl])
                        prev_spk[ec] = spk
                        continue
                    prev_spk[ec] = spk
                    # alternate DMA queues so the out transfers never serialize
                    dq = nc.sync if ec % 2 == 0 else nc.scalar
                    dq.dma_start(out_d[ec * P:(ec + 1) * P, xs], spk[:])

            for ti in range(T):
                xs = slice(ti * L, (ti + 1) * L)
                if ti > 0 and SCHED["po_first"]:
                    emit_o(ti - 1)
                for c in range(EC):
                    pso = ps.tile([P, L], F32, tag="mm512", name=f"pso{ti}_{c}")
                    nc.tensor.matmul(pso[:], kvt[ti * 4 + c][:], qT[c][:, xs],
                                     start=True, stop=True)
                    nc.scalar.copy(ao[c][:, xs], pso[:])
                if ti == 0 and SCHED["kv3"] == "ti0":
                    emit_kv(T - 1)
                if ti > 0 and not SCHED["po_first"]:
                    emit_o(ti - 1)
            emit_o(T - 1)

    nc.compile()
    _PROGRAM_CACHE["nc"] = nc
    return nc


def _rne11(a):
    """Round float32 array to 12-bit significands (RNE) - replicates the PE's
    fp32r operand rounding exactly."""
    m, e = np.frexp(np.asarray(a, np.float32).astype(np.float64))
    return np.ldexp(np.rint(m * 4096.0) / 4096.0, e).astype(np.float32)


def _e5m2(a):
    return np.asarray(a, np.float32).astype(ml_dtypes.float8_e5m2)


def kernel(**inputs):
    nc = _build_program()

    f64 = np.float64
    x = np.asarray(inputs["x"], np.float32)

    def bn_fold(g, b_, rm, rv):
        s = (g.astype(f64) / np.sqrt(rv.astype(f64) + EPS))
        bias = b_.astype(f64) - rm.astype(f64) * s
        return s, bias

    sq, bq = bn_fold(inputs["q_g"], inputs["q_b"], inputs["q_rm"], inputs["q_rv"])
    sk, bk = bn_fold(inputs["k_g"], inputs["k_b"], inputs["k_rm"], inputs["k_rv"])
    sv, bv = bn_fold(inputs["v_g"], inputs["v_b"], inputs["v_rm"], inputs["v_rv"])
    so, bo = bn_fold(inputs["o_g"], inputs["o_b"], inputs["o_rm"], inputs["o_rv"])
    C = HD ** -0.5
    # o path: out = bn(lin + o_bias) -> bias' = (o_bias - rm)*s + b
    bo = bo + inputs["o_bias"].astype(f64) * so

    wq = np.ascontiguousarray(inputs["q_w"].astype(f64).T).astype(np.float32)
    wk = np.ascontiguousarray((inputs["k_w"].astype(f64) * (C * sk)[:, None]).T).astype(np.float32)
    wv = np.ascontiguousarray((inputs["v_w"].astype(f64) * sv[:, None]).T).astype(np.float32)
    wo = np.ascontiguousarray(inputs["o_w"].astype(f64).T).astype(np.float32)
    kb_fold = (C * bk).astype(np.float32)
    vb_fold = bv.astype(np.float32)

    def wpack(w):
        return np.ascontiguousarray(np.concatenate(
            [w[dc * P:(dc + 1) * P, :] for dc in range(DC)], axis=1))

    cvars = (sq, bq, 1.0 - bo, 1.0 - 1.5 * bo, 1.0 - 1.75 * bo, 1.0 - 1.875 * bo)
    shared = {
        "wq": wpack(wq),
        "wq8": _e5m2(wpack(wq) / FP8_SCALE).reshape(P, DC, D),
        "wk": wpack(wk), "wv": wpack(wv),
        "wo": wpack(wo * so.astype(np.float32)[None, :]),
        "cpack": np.stack([np.asarray(v, f64).astype(np.float32).reshape(EC, P).T
                           for v in cvars], axis=1).reshape(P, 24),
        "kbb": np.ascontiguousarray(np.broadcast_to(kb_fold[None, :], (P, D))),
        "vthrp": np.ascontiguousarray(np.concatenate([
            np.broadcast_to((np.float32(1.0) - vb_fold)[None, :], (P, D)),
            np.broadcast_to((np.float32(-1.0) - vb_fold)[None, :], (P, D)),
        ], axis=1)),
    }

    in_maps = []
    for b in range(B):
        xT = x[b].reshape(NT, D).T                         # (D, NT) f32
        xl = _e5m2((xT - _rne11(xT)) * FP8_SCALE)
        # pack quarter-major, dc-major: xp[:, (q4*DC + dc)*L + t] = xT[dc-chunk, q4-quarter]
        xp = np.concatenate([xT[dc * P:(dc + 1) * P, q4 * L:(q4 + 1) * L]
                             for q4 in range(T) for dc in range(DC)], axis=1)
        xlp = np.concatenate([xl[dc * P:(dc + 1) * P, q4 * L:(q4 + 1) * L]
                              for q4 in range(T) for dc in range(DC)], axis=1)
        m = dict(shared)
        m["xp"] = np.ascontiguousarray(xp)
        m["xlp"] = np.ascontiguousarray(xlp).reshape(P, T * DC, L)
        in_maps.append(m)

    global _last_in_maps
    _last_in_maps = in_maps
    res = run_bass_kernel_spmd(nc, in_maps, core_ids=list(range(B)))
    outs = []
    for b in range(B):
        oT = res.results[b]["out"]                    # (D, NT) bf16
        outs.append(oT.reshape(D, T, L).transpose(1, 2, 0))
    return np.stack(outs).astype(np.float32)


if __name__ == "__main__":
    import importlib.util
    spec = importlib.util.spec_from_file_location("reference", "/root/problem/reference.py")
    ref = importlib.util.module_from_spec(spec)
    spec.loader.exec_module(ref)
    inp = {k: np.asarray(v) for k, v in ref.setup_inputs().items()}
    exp = np.asarray(ref.reference(**inp))
    act = kernel(**inp)
    rel = np.linalg.norm(act - exp) / np.linalg.norm(exp)
    print("flips:", int(np.sum(act != exp)), "/", exp.size)
    print("Relative error:", rel)


# revision 38
# speedup vs baseline: 1.3580x; 1.0331x over previous
"""Spikformer block (Q/K/V linear+BN+{LIF,ReLU,ternary} -> headwise linear attention
-> O linear+BN+LIF) on 8 TRN2 NeuronCores, data-parallel over batch.

Self-contained: hardcodes shapes; builds one SPMD Bass program; shards batch
across 8 cores; gathers/transposes on host.

Key algebra / precision plan:
  - attention has no softmax -> (q k^T) v reassociated as q (k^T v); per-head
    k^T v is 64x64, ~8x fewer MACs and no 512x512 attn matrix to evict.
  - all BatchNorms are eval-mode affine: folded into per-channel scale/bias on
    host.
  - matmul dtypes tuned to the LIF/ternary flip sensitivity against the
    rel-err budget (fp32r rounds operands to 12-bit significands (RNE) and
    streams at 1 cyc/row for free dim >= 256; flip counts per precision
    config measured with a bit-faithful numpy model against the reference):
      * Q linear: 2 passes (fp32r main + bf16 x-residual); the q-LIF
        threshold is the second most flip-sensitive spot, and the x-rounding
        term dominates its flips.
      * K linear: 1 fp32r pass (errors wash out in the k^T v sum)
      * V linear: 1 fp32r pass (ternary threshold flips stay in budget)
      * O linear: 1 fp32r pass
      * attention: fp16 throughout (q binary and v ternary are exact in
        fp16; the k/kv quantization adds ~1e-4 relative, tolerable).
  - O-path bias is folded into the act-engine bias adds (steps 0..2) and into
    the precomputed final-step threshold r3t (last step) - no bias matmuls.
  - k^T v computed as one 128-wide matmul per token chunk covering 2 heads;
    the cross-head off-diagonal blocks are garbage but never read: only the
    two diagonal 64x64 blocks are copied into pre-zeroed kv tiles. kv runs
    one quarter behind V so the V thresholds (DVE) stay off the critical
    path, and the copies land on the act engine which idles during V.
  - LIF membrane updates run as in-place DVE scalar_tensor_tensor ops; Pool
    only computes spikes (is_ge), keeping every engine under the PE time.
"""
import sys
for p in ("/opt/trn_rl_repo",):
    if p not in sys.path:
        sys.path.insert(0, p)

import numpy as np
import ml_dtypes

import concourse.bass as bass
import concourse.bacc as bacc
import concourse.mybir as mybir
import concourse.tile as tile
from concourse.bass_utils import run_bass_kernel_spmd

B, T, L, D, H = 8, 4, 512, 512, 8
HD = D // H
NT = T * L            # 2048 tokens per core
P = 128
EC = D // P           # 4 e-chunks
DC = D // P           # 4 d-chunks
TC = NT // P          # 16 token chunks
EPS = 1e-5
F32 = mybir.dt.float32
F32R = mybir.dt.float32r
BF16 = mybir.dt.bfloat16
FP16 = mybir.dt.float16
FP8 = mybir.dt.float8e5
FP8_SCALE = 32.0
F = mybir.ActivationFunctionType
ALU = mybir.AluOpType

_PROGRAM_CACHE = {}
_last_in_maps = None

# structural scheduling choices (CoreSim-swept)
SCHED = {
    "v3_first": False,     # emit V(3) before Q(3) in the last phase-1 window
    "kv3": "ph1",          # 'ph1' (end of phase 1) or 'ti0' (in phase-2 ti0)
    "po_first": False,     # emit_o(ti-1) before pso(ti) in phase 2
    "ao_ti0_act": False,   # ti0 ao copies all on ACT
}


def _build_program():
    if "nc" in _PROGRAM_CACHE:
        return _PROGRAM_CACHE["nc"]

    nc = bacc.Bacc("TRN2", target_bir_lowering=False, debug=False, num_devices=8)

    dram = {}
    # x and weights are packed on host into [128, n*512] layouts so each
    # SBUF tile fills with a single DMA (HWDGE slots are ~630ns each).
    dram["xp"] = nc.dram_tensor("xp", [P, T * DC * L], F32R, kind="ExternalInput")
    dram["xlp"] = nc.dram_tensor("xlp", [P, T * DC, L], FP8, kind="ExternalInput")
    for w in ("wq", "wk", "wv", "wo"):
        dram[w] = nc.dram_tensor(w, [P, DC * D], F32R, kind="ExternalInput")
    dram["wq8"] = nc.dram_tensor("wq8", [P, DC, D], FP8, kind="ExternalInput")
    dram["cpack"] = nc.dram_tensor("cpack", [P, 24], F32, kind="ExternalInput")
    dram["kbb"] = nc.dram_tensor("kbb", [P, D], F32, kind="ExternalInput")
    dram["vthrp"] = nc.dram_tensor("vthrp", [P, 2 * D], F32, kind="ExternalInput")
    out_d = nc.dram_tensor("out", [D, NT], BF16, kind="ExternalOutput")

    with tile.TileContext(nc) as tc_:
        with tc_.tile_pool(name="sb", bufs=1) as sb, \
             tc_.tile_pool(name="sc", bufs=3) as sc, \
             tc_.tile_pool(name="sp8", bufs=6) as sp8, \
             tc_.tile_pool(name="ps", bufs=5, space="PSUM") as ps, \
             tc_.tile_pool(name="pk", bufs=3, space="PSUM") as pk:

            # ---------- persistent SBUF tiles ----------
            # x per quarter, dc-major columns: xq[q4][:, dc*L + t]
            xq = [sb.tile([P, DC * L], F32R, tag="xa", bufs=DC, name=f"xq{i}") for i in range(T)]
            xlq = [sb.tile([P, DC, L], FP8, tag="xl", bufs=DC, name=f"xlq{i}") for i in range(T)]
            wts = {}
            for w in ("wq", "wk", "wv", "wo"):
                wts[w] = sb.tile([P, DC * D], F32R, tag=w, name=w)
            wts["wq8"] = sb.tile([P, DC, D], FP8, tag="wq8", name="wq8")
            qT = [sb.tile([P, NT], FP16, tag="qT", bufs=EC, name=f"qT{i}") for i in range(EC)]
            k16 = [sb.tile([P, D], FP16, tag="k16", bufs=TC, name=f"k16_{i}") for i in range(TC)]
            v16 = [sb.tile([P, D], FP16, tag="v16", bufs=TC, name=f"v16_{i}") for i in range(TC)]
            kvt = [sb.tile([P, P], FP16, tag="kvt", bufs=TC, name=f"kvt_{i}") for i in range(TC)]
            # attention output [e, t] fp32r (dedicated tiles; the fp8/fp16
            # shrinkage leaves enough SBUF, enabling early qkv emission)
            ao = [sb.tile([P, NT], F32R, tag="aot", bufs=DC, name=f"ao{i}") for i in range(DC)]
            memq = [sb.tile([P, L], F32, tag="memq", bufs=EC, name=f"memq{i}") for i in range(EC)]
            memo = [sb.tile([P, L], F32, tag="memo", bufs=EC, name=f"memo{i}") for i in range(EC)]
            # r3t is only live in phase 2; reuse the memq buffers (dead after
            # phase 1b) by allocating on the same tag ring.
            r3t = [sb.tile([P, L], F32, tag="memq", bufs=EC, name=f"r3t{i}") for i in range(EC)]
            cpack = sb.tile([P, 24], F32, tag="cst")
            consts = {v_: [cpack[:, (vi * 4 + i):(vi * 4 + i + 1)] for i in range(EC)]
                      for vi, v_ in enumerate(("qs", "qb", "othr0", "othr1", "othr2", "or3c"))}
            kbbt = sb.tile([P, D], F32, tag="kbb")
            kbb = kbbt[:]
            vthrp = sb.tile([P, 2 * D], F32, tag="thr")
            vthr1 = vthrp[:, 0:D]
            vthr2 = vthrp[:, D:2 * D]

            # ---------- loads ----------
            # head: first K matmul needs full wk + full xq0; split them across
            # four DMA queues (SP/act/pool/vector) so both land ~2x sooner.
            # Everything else streams on the SP queue in consumption order.
            W = DC * L
            nc.sync.dma_start(wts["wk"][:, 0:2 * D], dram["wk"][:, 0:2 * D])
            nc.scalar.dma_start(wts["wk"][:, 2 * D:DC * D], dram["wk"][:, 2 * D:DC * D])
            nc.gpsimd.dma_start(xq[0][:, 0:W // 2], dram["xp"][:, 0:W // 2])
            nc.sync.dma_start(xq[0][:, W // 2:W], dram["xp"][:, W // 2:W])

            # kv tiles: zero once; only diagonal 64x64 blocks are ever written
            for i in range(TC):
                nc.gpsimd.memset(kvt[i][:], 0.0)

            def ldx(q4, lo=False):
                if lo:
                    nc.sync.dma_start(xlq[q4][:], dram["xlp"][:, q4 * DC:(q4 + 1) * DC, :])
                    return
                for hh in range(2):
                    nc.sync.dma_start(xq[q4][:, hh * W // 2:(hh + 1) * W // 2],
                                      dram["xp"][:, q4 * W + hh * W // 2:q4 * W + (hh + 1) * W // 2])

            # Q-phase operands ride the act queue (idle until the K relus),
            # landing well before the K->Q boundary; everything else streams
            # on SP behind the K-phase x quarters.
            nc.scalar.dma_start(wts["wq"][:], dram["wq"][:])
            nc.scalar.dma_start(wts["wq8"][:], dram["wq8"][:])
            nc.scalar.dma_start(xlq[0][:], dram["xlp"][:, 0:DC, :])
            nc.sync.dma_start(kbbt[:], dram["kbb"][:])
            ldx(1)
            ldx(2)
            ldx(3)
            nc.sync.dma_start(cpack[:], dram["cpack"][:])
            nc.sync.dma_start(wts["wv"][:], dram["wv"][:])
            ldx(1, lo=True)
            nc.sync.dma_start(vthrp[:], dram["vthrp"][:])
            ldx(2, lo=True)
            ldx(3, lo=True)
            nc.sync.dma_start(wts["wo"][:], dram["wo"][:])

            # ---------- phase 1a: K tiles (relu'd, fp16) ----------
            for tc2 in range(TC):
                q4k, mc4 = tc2 // 4, tc2 % 4
                pkv = ps.tile([P, D], F32, tag="mm512", name=f"pk{tc2}")
                for dc in range(DC):
                    nc.tensor.matmul(pkv[:], xq[q4k][:, dc * L + mc4 * P:dc * L + (mc4 + 1) * P],
                                     wts["wk"][:, dc * D:(dc + 1) * D],
                                     start=(dc == 0), stop=(dc == DC - 1))
                kf = sc.tile([P, D], F32, tag="kf", name=f"kf{tc2}")
                nc.vector.tensor_tensor(kf[:], pkv[:], kbb, ALU.add)
                nc.scalar.activation(k16[tc2][:], kf[:], F.Relu)

            # ---------- phase 1b: interleaved Q (2-pass, LIF over T) and V
            # quarters. A Q+V window has ~11.1us of PE work against ~7.1us of
            # DVE work, absorbing the V thresholds' DVE deficit; the kv
            # matmuls trail the V quarter by one window so the v16 threshold
            # chains are never on the PE critical path.
            def emit_q(ti):
                xs = slice(ti * L, (ti + 1) * L)
                for ec in range(EC):
                    pq = ps.tile([P, L], F32, tag="mm512", name=f"pq{ti}_{ec}")
                    for dc in range(DC):
                        nc.tensor.matmul(pq[:], wts["wq"][:, dc * D + ec * P:dc * D + (ec + 1) * P],
                                         xq[ti][:, dc * L:(dc + 1) * L],
                                         start=(dc == 0), stop=False)
                    for dcp in (0, 2):
                        nc.tensor.matmul(pq[:], wts["wq8"][:, dcp:dcp + 2, ec * P:(ec + 1) * P],
                                         xlq[ti][:, dcp:dcp + 2, :],
                                         start=False, stop=(dcp == 2),
                                         perf_mode=mybir.MatmulPerfMode.DoubleRow)
                    if ti == 0:
                        nc.scalar.activation(memq[ec][:], pq[:], F.Identity,
                                             bias=consts["qb"][ec], scale=consts["qs"][ec])
                    else:
                        u = sc.tile([P, L], F32, tag="u", name=f"u{ti}_{ec}")
                        nc.scalar.activation(u[:], pq[:], F.Identity,
                                             bias=consts["qb"][ec], scale=consts["qs"][ec])
                        nc.vector.scalar_tensor_tensor(memq[ec][:], memq[ec][:], 0.5, u[:],
                                                       ALU.mult, ALU.add)
                        nc.gpsimd.tensor_sub(memq[ec][:], memq[ec][:],
                                             qT[ec][:, (ti - 1) * L:ti * L])
                    nc.gpsimd.tensor_scalar(qT[ec][:, xs], memq[ec][:], 1.0, None, ALU.is_ge)

            def emit_v(q4):
                for mc in range(4):
                    tc2 = q4 * 4 + mc
                    pvv = ps.tile([P, D], F32, tag="mm512", name=f"pv{tc2}")
                    for dc in range(DC):
                        nc.tensor.matmul(pvv[:], xq[q4][:, dc * L + mc * P:dc * L + (mc + 1) * P],
                                         wts["wv"][:, dc * D:(dc + 1) * D],
                                         start=(dc == 0), stop=(dc == DC - 1))
                    t1 = sc.tile([P, D], FP16, tag="t1", name=f"t1_{tc2}")
                    t2 = sc.tile([P, D], FP16, tag="t2", name=f"t2_{tc2}")
                    nc.vector.tensor_tensor(t1[:], pvv[:], vthr1, ALU.is_ge)
                    nc.vector.tensor_tensor(t2[:], pvv[:], vthr2, ALU.is_le)
                    nc.gpsimd.tensor_sub(v16[tc2][:], t1[:], t2[:])

            def emit_kv(ti):
                # head pair (2c, 2c+1) == e-chunk c; diagonal 64x64 blocks only
                pkvs = []
                for c in range(EC):
                    es = slice(c * P, (c + 1) * P)
                    pkv64 = pk.tile([P, P], F32, tag="kv64", name=f"pkv{ti}_{c}")
                    for mc in range(4):
                        tc2 = ti * 4 + mc
                        nc.tensor.matmul(pkv64[:], k16[tc2][:, es], v16[tc2][:, es],
                                         start=(mc == 0), stop=(mc == 3))
                    pkvs.append(pkv64)
                for c in range(EC):
                    kv = kvt[ti * 4 + c]
                    nc.scalar.copy(kv[0:HD, 0:HD], pkvs[c][0:HD, 0:HD])
                    nc.scalar.copy(kv[HD:P, HD:P], pkvs[c][HD:P, HD:P])

            def emit_qkv(ti):
                xs = slice(ti * L, (ti + 1) * L)
                for c in range(EC):
                    pso = ps.tile([P, L], F32, tag="mm512", name=f"pso{ti}_{c}")
                    nc.tensor.matmul(pso[:], kvt[ti * 4 + c][:], qT[c][:, xs],
                                     start=True, stop=True)
                    nc.scalar.copy(ao[c][:, xs], pso[:])

            # qkv(i-2) rides window i: its kvt copies got a full window of
            # slack, and its ao copies land long before emit_o needs them
            for i in range(T):
                emit_q(i)
                if i >= 2:
                    emit_qkv(i - 2)
                emit_v(i)
                if i > 0:
                    emit_kv(i - 1)

            # ---------- phase 2: attention q(kv) and O-linear per ti --------
            prev_spk = [None] * EC

            def emit_o(ti, ecs=range(EC)):
                # wo is pre-scaled by the BN scale on host. The per-step bias
                # ob is never added: the membrane is tracked as
                #   m~_t = mem_t - c_t,   c_t = sum_{s<=t} 0.5^(t-s) ob
                # so updates are pure stt ops reading the PSUM po directly,
                # and spikes compare against per-channel thresholds 1 - c_t.
                xs = slice(ti * L, (ti + 1) * L)
                final = (ti == T - 1)
                for ec in ecs:
                    po = ps.tile([P, L], F32, tag="mm512")
                    for dc in range(DC):
                        nc.tensor.matmul(po[:], wts["wo"][:, dc * D + ec * P:dc * D + (ec + 1) * P],
                                         ao[dc][:, xs],
                                         start=(dc == 0), stop=(dc == DC - 1))
                    spk = sp8.tile([P, L], BF16, tag="ospk")
                    if ti == 0:
                        # m~0 = po; spike straight off PSUM, then build the
                        # reset-applied state memo = po - 2 spk in one stt
                        nc.vector.tensor_scalar(spk[:], po[:], consts["othr0"][ec], None,
                                                ALU.is_ge)
                        nc.vector.scalar_tensor_tensor(memo[ec][:], spk[:], -2.0,
                                                       po[:], ALU.mult, ALU.add)
                    elif not final:
                        nc.vector.scalar_tensor_tensor(memo[ec][:], memo[ec][:], 0.5,
                                                       po[:], ALU.mult, ALU.add)
                        thr = consts["othr1" if ti == 1 else "othr2"][ec]
                        nc.gpsimd.tensor_scalar(spk[:], memo[ec][:], thr, None, ALU.is_ge)
                        if ti == T - 2:
                            # r3t = (1 - c3) - m~2/2 + spk2, all on Pool
                            nc.gpsimd.tensor_scalar(r3t[ec][:], memo[ec][:], -0.5,
                                                    consts["or3c"][ec], ALU.mult, ALU.add)
                            nc.gpsimd.tensor_tensor(r3t[ec][:], r3t[ec][:], spk[:], ALU.add)
                        else:
                            # reset in place: memo -= 2 spk; split engines
                            if ec < 2:
                                nc.gpsimd.tensor_sub(memo[ec][:], memo[ec][:], spk[:])
                                nc.gpsimd.tensor_sub(memo[ec][:], memo[ec][:], spk[:])
                            else:
                                nc.vector.scalar_tensor_tensor(memo[ec][:], spk[:], -2.0,
                                                               memo[ec][:], ALU.mult, ALU.add)
                    else:
                        # final step: entire LIF update folded into one compare
                        # against the precomputed r3t (which includes the bias);
                        # done in halves so each half's out DMA starts sooner
                        for h in range(2):
                            hl = slice(h * L // 2, (h + 1) * L // 2)
                            hx = slice(ti * L + h * L // 2, ti * L + (h + 1) * L // 2)
                            nc.vector.tensor_tensor(spk[:, hl], po[:, hl],
                                                    r3t[ec][:, hl], ALU.is_ge)
                            dq = nc.sync if h == 0 else nc.scalar
                            dq.dma_start(out_d[ec * P:(ec + 1) * P, hx], spk[:, hl])
                        prev_spk[ec] = spk
                        continue
                    prev_spk[ec] = spk
                    # alternate DMA queues so the out transfers never serialize
                    dq = nc.sync if ec % 2 == 0 else nc.scalar
                    dq.dma_start(out_d[ec * P:(ec + 1) * P, xs], spk[:])

            # emit_o(0) fills the PE while V(3)'s threshold chains drain,
            # unblocking kv(3); qkv(3) sits between o1/o2 so o2's r3t chain
            # gets extra PE cover before the final compares
            emit_qkv(2)
            emit_o(0)
            emit_kv(T - 1)
            emit_o(1)
            emit_qkv(T - 1)
            emit_o(2)
            emit_o(T - 1)

    nc.compile()
    _PROGRAM_CACHE["nc"] = nc
    return nc


def _rne11(a):
    """Round float32 array to 12-bit significands (RNE) - replicates the PE's
    fp32r operand rounding exactly."""
    m, e = np.frexp(np.asarray(a, np.float32).astype(np.float64))
    return np.ldexp(np.rint(m * 4096.0) / 4096.0, e).astype(np.float32)


def _e5m2(a):
    return np.asarray(a, np.float32).astype(ml_dtypes.float8_e5m2)


def kernel(**inputs):
    nc = _build_program()

    f64 = np.float64
    x = np.asarray(inputs["x"], np.float32)

    def bn_fold(g, b_, rm, rv):
        s = (g.astype(f64) / np.sqrt(rv.astype(f64) + EPS))
        bias = b_.astype(f64) - rm.astype(f64) * s
        return s, bias

    sq, bq = bn_fold(inputs["q_g"], inputs["q_b"], inputs["q_rm"], inputs["q_rv"])
    sk, bk = bn_fold(inputs["k_g"], inputs["k_b"], inputs["k_rm"], inputs["k_rv"])
    sv, bv = bn_fold(inputs["v_g"], inputs["v_b"], inputs["v_rm"], inputs["v_rv"])
    so, bo = bn_fold(inputs["o_g"], inputs["o_b"], inputs["o_rm"], inputs["o_rv"])
    C = HD ** -0.5
    # o path: out = bn(lin + o_bias) -> bias' = (o_bias - rm)*s + b
    bo = bo + inputs["o_bias"].astype(f64) * so

    wq = np.ascontiguousarray(inputs["q_w"].astype(f64).T).astype(np.float32)
    wk = np.ascontiguousarray((inputs["k_w"].astype(f64) * (C * sk)[:, None]).T).astype(np.float32)
    wv = np.ascontiguousarray((inputs["v_w"].astype(f64) * sv[:, None]).T).astype(np.float32)
    wo = np.ascontiguousarray(inputs["o_w"].astype(f64).T).astype(np.float32)
    kb_fold = (C * bk).astype(np.float32)
    vb_fold = bv.astype(np.float32)

    def wpack(w):
        return np.ascontiguousarray(np.concatenate(
            [w[dc * P:(dc + 1) * P, :] for dc in range(DC)], axis=1))

    cvars = (sq, bq, 1.0 - bo, 1.0 - 1.5 * bo, 1.0 - 1.75 * bo, 1.0 - 1.875 * bo)
    shared = {
        "wq": wpack(wq),
        "wq8": _e5m2(wpack(wq) / FP8_SCALE).reshape(P, DC, D),
        "wk": wpack(wk), "wv": wpack(wv),
        "wo": wpack(wo * so.astype(np.float32)[None, :]),
        "cpack": np.stack([np.asarray(v, f64).astype(np.float32).reshape(EC, P).T
                           for v in cvars], axis=1).reshape(P, 24),
        "kbb": np.ascontiguousarray(np.broadcast_to(kb_fold[None, :], (P, D))),
        "vthrp": np.ascontiguousarray(np.concatenate([
            np.broadcast_to((np.float32(1.0) - vb_fold)[None, :], (P, D)),
            np.broadcast_to((np.float32(-1.0) - vb_fold)[None, :], (P, D)),
        ], axis=1)),
    }

    in_maps = []
    for b in range(B):
        xT = x[b].reshape(NT, D).T                         # (D, NT) f32
        xl = _e5m2((xT - _rne11(xT)) * FP8_SCALE)
        # pack quarter-major, dc-major: xp[:, (q4*DC + dc)*L + t] = xT[dc-chunk, q4-quarter]
        xp = np.concatenate([xT[dc * P:(dc + 1) * P, q4 * L:(q4 + 1) * L]
                             for q4 in range(T) for dc in range(DC)], axis=1)
        xlp = np.concatenate([xl[dc * P:(dc + 1) * P, q4 * L:(q4 + 1) * L]
                              for q4 in range(T) for dc in range(DC)], axis=1)
        m = dict(shared)
        m["xp"] = np.ascontiguousarray(xp)
        m["xlp"] = np.ascontiguousarray(xlp).reshape(P, T * DC, L)
        in_maps.append(m)

    global _last_in_maps
    _last_in_maps = in_maps
    res = run_bass_kernel_spmd(nc, in_maps, core_ids=list(range(B)))
    outs = []
    for b in range(B):
        oT = res.results[b]["out"]                    # (D, NT) bf16
        outs.append(oT.reshape(D, T, L).transpose(1, 2, 0))
    return np.stack(outs).astype(np.float32)


if __name__ == "__main__":
    import importlib.util
    spec = importlib.util.spec_from_file_location("reference", "/root/problem/reference.py")
    ref = importlib.util.module_from_spec(spec)
    spec.loader.exec_module(ref)
    inp = {k: np.asarray(v) for k, v in ref.setup_inputs().items()}
    exp = np.asarray(ref.reference(**inp))
    act = kernel(**inp)
    rel = np.linalg.norm(act - exp) / np.linalg.norm(exp)
    print("flips:", int(np.sum(act != exp)), "/", exp.size)
    print("Relative error:", rel)
